# revision 50
# baseline (speedup 1.0000x reference)
"""Multi-head causal attention (QKV proj + RoPE + softmax) on 8 TRN2 NeuronCores.

Sharding: batch 4-way x head-group 2-way -> each core handles 1 batch and 8
contiguous heads (512 output channels). No collectives; host gathers slices.

Per-core algorithm (all matmul compute in bf16, fp32 PSUM accumulation):
  - host passes x.T (q/k/v of its batch, transposed to [emb, seq]) and W.T
    shards so every matmul contracts over the partition dim without on-device
    transposes.
  - q/k weights are row-permuted per head into [even dims | odd dims] so RoPE
    becomes: rot = x*cs + swap32(x)*sn, where swap32 is an SBUF partition-block
    swap done by DMA. The per-head dim permutation cancels in q.k dot products.
  - q/k biases are per-partition columns folded into the PSUM eviction
    (tensor_scalar add); the v bias is applied on host: P@(V+b) = P@V + l*b.
  - scores are computed transposed, S_T[k, q] = kh_T.T @ qh_T (K=64
    contraction; the A/B heads of a 128-row tile are emitted adjacently so
    they run concurrently on PE row groups).
  - softmax: exp on ScalarE from PSUM (no max subtraction: |scores| <= ~5 by
    construction); causal mask multiplies on DVE for the diagonal tiles only.
  - attnT[d, q] = sum_kt V_tile[k,d|1].T @ P_T[k, q] -- a ones-column appended
    to V makes row 64 the softmax denominator for free.
  - unnormalized attnT and the denominator row go to HBM via one SBUF staging
    copy; division + final transpose + v-bias happen on host.

Scheduling (the critical part): ScalarE exp is the busiest engine
(~160us); the kernel streams all causal score tiles ("instances") through
ScalarE continuously starting ~9us in.  q/k projections are chunked per
(m-tile, 512-seq chunk) with per-chunk RoPE so head-pair 0 is ready almost
immediately; v-projection chunks and the remaining q/k chunks are emitted as
background PE work on a static pacing map between score matmuls.  exp output
goes to a 12-slot probs ring; the attn@V matmuls lag 8 instances behind exp
so PE never blocks ScalarE.  Diagonal tiles stream only the valid query
columns (saves ~15% on every engine).
"""

import sys
import types
from collections import defaultdict, deque

import numpy as np
import ml_dtypes

BF16 = ml_dtypes.bfloat16
F8 = ml_dtypes.float8_e4m3
SEQ, EMB, NHEADS, BATCH = 2048, 1024, 16, 4
HD, HALF = 64, 32
HPC = 8          # heads per core
DH = 512         # output dims per core
NE = EMB // 128  # 8 contraction tiles
NT = 4           # head-pair (128-row) dout tiles
NKT = SEQ // 128  # 16 key tiles
NQC = SEQ // 512  # 4 query chunks
SC = 512
NSLOT = 10       # probs ring slots
VLAG = 7         # attn@V matmul lag behind exp, in kt-instances
MOFF = [0, 512, 896, 1152]  # packed mask col offsets per diagonal d


def _install_ntff_shim():
    """The image's antenv lacks axon_hooks; synthesize it from trn_agent_boot
    so run_bass_kernel_spmd(trace=True) can profile. Harmless if unused."""
    try:
        import antenv.axon_hooks  # noqa: F401
        return
    except ImportError:
        pass
    try:
        from trn_agent_boot.trn_boot import _ntff_profile_via_ctypes
        import antenv
    except ImportError:
        return
    hook = _ntff_profile_via_ctypes("/opt/axon/libaxon_pjrt.so")
    mod = types.ModuleType("antenv.axon_hooks")
    mod.get_axon_ntff_profile_hook = lambda: hook
    mod.set_axon_ntff_profile_hook = lambda h: None
    sys.modules["antenv.axon_hooks"] = mod
    antenv.axon_hooks = mod


_built = {}


def build(causal=True):
    if causal in _built:
        return _built[causal]
    import concourse.mybir as mybir
    import concourse.tile as tile
    from concourse import bacc

    f32 = mybir.dt.float32
    bf = mybir.dt.bfloat16
    f8 = mybir.dt.float8e4
    DR = mybir.MatmulPerfMode.DoubleRow
    EXP = mybir.ActivationFunctionType.Exp
    MUL = mybir.AluOpType.mult
    ADD = mybir.AluOpType.add

    nc = bacc.Bacc(None, target_bir_lowering=False, debug=False)
    with tile.TileContext(nc) as tc:
        with tc.tile_pool(name="dram", bufs=1, space="DRAM") as dram:
            # x and W arrive partition-major ([128, NE*cols]) so one DMA
            # trigger loads a full [128, NE, colrange] slab
            xq_d = dram.tile([128, NE * SEQ], bf, kind="ExternalInput", name="xq", uniquify=False)
            xk_d = dram.tile([128, NE * SEQ], f8, kind="ExternalInput", name="xk", uniquify=False)
            xv_d = dram.tile([128, NE * SEQ], bf, kind="ExternalInput", name="xv", uniquify=False)
            wq_d = dram.tile([128, NE * DH], bf, kind="ExternalInput", name="wq", uniquify=False)
            wk_d = dram.tile([128, NE * DH], f8, kind="ExternalInput", name="wk", uniquify=False)
            wv_d = dram.tile([128, NE * DH], bf, kind="ExternalInput", name="wv", uniquify=False)
            # bias(8xf32 as 16xbf16 raw) | cs | sn in one small early DMA;
            # mask (single copy, applied per-half) in another
            msc_d = dram.tile([128, 16 + 2 * SEQ], bf, kind="ExternalInput",
                              name="msc", uniquify=False)
            mk_d = dram.tile([128, 1280], bf, kind="ExternalInput",
                             name="msk", uniquify=False)
            outT_d = dram.tile([DH, SEQ], f32, kind="ExternalOutput", name="outT", uniquify=False)
            l_d = dram.tile([HPC, SEQ], f32, kind="ExternalOutput", name="lsum", uniquify=False)

            with tc.tile_pool(name="const", bufs=1) as cp, \
                 tc.tile_pool(name="xv", bufs=3) as xvp, \
                 tc.tile_pool(name="rope", bufs=2) as rp, \
                 tc.tile_pool(name="ostage", bufs=3) as op, \
                 tc.tile_pool(name="pp", bufs=2, space="PSUM") as pp, \
                 tc.tile_pool(name="sp", bufs=2, space="PSUM") as sp, \
                 tc.tile_pool(name="tA", bufs=1, space="PSUM") as ptA, \
                 tc.tile_pool(name="tB", bufs=1, space="PSUM") as ptB:

                qh = cp.tile([128, NT, SEQ], bf, name="qh")
                kh = cp.tile([128, NT, SEQ], bf, name="kh")
                vsb = cp.tile([128, NKT, HPC * 65], bf, name="vsb")
                probs = cp.tile([128, NSLOT, 2, SC], bf, name="probs")
                w_sb = {"q": cp.tile([128, NT, NE, 128], bf, name="w_q"),
                        "k": cp.tile([128, NT, NE, 128], f8, name="w_k"),
                        "v": cp.tile([128, NE, DH], bf, name="w_v")}
                msc = cp.tile([128, 16 + 2 * SEQ], bf, name="msc")
                bb = msc[:, 0:16].bitcast(f32)          # [128, 8] f32
                b_sb = {"q": bb[:, 0:NT], "k": bb[:, NT:2 * NT]}
                cs = msc[:, 16:16 + SEQ]
                sn = msc[:, 16 + SEQ:16 + 2 * SEQ]
                msk = cp.tile([128, 1280], bf, name="mskt")
                scr = cp.tile([128, 16], bf, name="scr")
                scrW = cp.tile([128, 64], bf, name="scrW")
                scrX = cp.tile([128, 512], bf, name="scrX")

                # ---- warmup: ACT table preload + PE HAM spin-up (independent
                # scratch tiles so the MMs don't serialize behind ScalarE)
                nc.vector.memset(scr[:, :], 0.0)
                nc.vector.memset(scrW[:, :], 0.0)
                nc.vector.memset(scrX[:, :], 0.0)
                nc.scalar.activation(scr[:, 0:8], scr[:, 8:16], EXP)
                wps = pp.tile([128, SC], f32, tag="p", name="warm")
                for r in range(10):
                    nc.tensor.matmul(wps[0:64, :], scrW[:, :], scrX[:, :],
                                     start=True, stop=True)

                # ---- input loads: ONE queue (sync), strict priority order so
                # DMA-engine bandwidth serves the critical path first. x
                # tensors arrive sc-major, W m-major: every load is one
                # fully-contiguous slab. ----
                xq_v = xq_d[:, :].rearrange("p (c e s) -> p c e s", c=NQC, e=NE)
                xk_v = xk_d[:, :].rearrange("p (c e s) -> p c e s", c=NQC, e=NE)
                xv_v = xv_d[:, :].rearrange("p (c e s) -> p c e s", c=NQC, e=NE)
                wq_v = wq_d[:, :].rearrange("p (m e c) -> p m e c", m=NT, e=NE)
                wk_v = wk_d[:, :].rearrange("p (m e c) -> p m e c", m=NT, e=NE)
                wv_v = wv_d[:, :].rearrange("p (e c) -> p e c", e=NE)

                xq_sb = cp.tile([128, NQC, NE, SC], bf, name="xq_sb")
                xk_sb = cp.tile([128, NQC, NE, SC], f8, name="xk_sb")
                xv_sc = {}

                def load_x_sc(dst, srcv, sc):
                    nc.sync.dma_start(out=dst[:, sc, :, :], in_=srcv[:, sc, :, :])

                def load_xv_sc(sc):
                    t = xvp.tile([128, NE, SC], bf, tag="xv", name=f"xv{sc}")
                    nc.sync.dma_start(out=t[:, :, :], in_=xv_v[:, sc, :, :])
                    xv_sc[sc] = t

                nc.sync.dma_start(out=w_sb["q"][:, 0, :, :], in_=wq_v[:, 0, :, :])
                load_x_sc(xq_sb, xq_v, 0)
                nc.sync.dma_start(out=msc[:, :], in_=msc_d[:, :])
                nc.sync.dma_start(out=w_sb["k"][:, 0, :, :], in_=wk_v[:, 0, :, :])
                load_x_sc(xk_sb, xk_v, 0)
                nc.sync.dma_start(out=msk[:, :], in_=mk_d[:, :])
                load_x_sc(xq_sb, xq_v, 1)
                load_x_sc(xk_sb, xk_v, 1)
                nc.sync.dma_start(out=w_sb["v"][:, :, :], in_=wv_v[:, :, :])
                load_xv_sc(0)
                load_x_sc(xq_sb, xq_v, 2)
                load_x_sc(xk_sb, xk_v, 2)
                load_xv_sc(1)
                load_x_sc(xq_sb, xq_v, 3)
                load_x_sc(xk_sb, xk_v, 3)
                nc.sync.dma_start(out=w_sb["q"][:, 1:NT, :, :], in_=wq_v[:, 1:NT, :, :])
                nc.sync.dma_start(out=w_sb["k"][:, 1:NT, :, :], in_=wk_v[:, 1:NT, :, :])
                load_xv_sc(2)
                load_xv_sc(3)
                # only the ones-columns (col 64 of each 65-block) need init;
                # the v evictions overwrite the 64 data columns of every block
                nc.vector.memset(
                    vsb[:, :, :].rearrange("p k (h u) -> p k h u", u=65)[:, :, :, 64:65],
                    1.0)

                # ---- emitters ----
                def qk_chunk(which, m, sc):
                    # k: fp8 DoubleRow, 2 e-tiles per matmul (virtual K=256).
                    # q: bf16 col-split pairs (fp8 on both q and k pushes the
                    # softmax error past the gate; one side is safe).
                    x_sb = xq_sb if which == "q" else xk_sb
                    dst = qh if which == "q" else kh
                    ps = pp.tile([128, SC], f32, tag="p", name=f"pp{which}{m}{sc}")
                    if which == "k":
                        for ep in range(NE // 2):
                            nc.tensor.matmul(ps[:, :],
                                             w_sb["k"][:, m, 2 * ep:2 * ep + 2, :],
                                             x_sb[:, sc, 2 * ep:2 * ep + 2, :],
                                             start=(ep == 0),
                                             stop=(ep == NE // 2 - 1),
                                             perf_mode=DR)
                    else:
                        for e in range(NE):
                            nc.tensor.matmul(ps[0:64, :],
                                             w_sb["q"][:, m, e, 0:64],
                                             x_sb[:, sc, e, :],
                                             start=(e == 0), stop=(e == NE - 1))
                            nc.tensor.matmul(ps[64:128, :],
                                             w_sb["q"][:, m, e, 64:128],
                                             x_sb[:, sc, e, :],
                                             start=(e == 0), stop=(e == NE - 1))
                    tmp = rp.tile([128, SC], bf, tag="tmp", bufs=3, name=f"t{which}{m}{sc}")
                    nc.vector.tensor_scalar_add(tmp[:, :], ps[:, :],
                                                b_sb[which][:, m:m + 1])
                    tsw = rp.tile([128, SC], bf, tag="tsw", name=f"w{which}{m}{sc}")
                    # m0 swaps ride the gpsimd queue (sync is busy loading);
                    # later ones use sync (gpsimd carries rope+output traffic)
                    dq = nc.gpsimd if m == 0 else nc.sync
                    for blk in range(4):
                        s = blk ^ 1
                        dq.dma_start(out=tsw[blk * 32:(blk + 1) * 32, :],
                                     in_=tmp[s * 32:(s + 1) * 32, :])
                    m2 = rp.tile([128, SC], bf, tag="m2", name=f"m{which}{m}{sc}")
                    dsl = dst[:, m, sc * SC:(sc + 1) * SC]
                    nc.vector.tensor_tensor(dsl, tmp[:, :],
                                            cs[:, sc * SC:(sc + 1) * SC], MUL)
                    nc.vector.tensor_tensor(m2[:, :], tsw[:, :],
                                            sn[:, sc * SC:(sc + 1) * SC], MUL)
                    nc.vector.tensor_tensor(dsl, dsl, m2[:, :], ADD)

                def v_st(st):
                    sc, o = st // 4, (st % 4) * 128
                    xt = xv_sc[sc]
                    ps = pp.tile([128, SC], f32, tag="p", name=f"ppv{st}")
                    for e in range(NE):
                        nc.tensor.matmul(ps[0:64, :], xt[:, e, o:o + 64],
                                         w_sb["v"][:, e, :],
                                         start=(e == 0), stop=(e == NE - 1))
                        nc.tensor.matmul(ps[64:128, :], xt[:, e, o + 64:o + 128],
                                         w_sb["v"][:, e, :],
                                         start=(e == 0), stop=(e == NE - 1))
                    nc.vector.tensor_copy(
                        vsb[:, st, :].rearrange("p (h u) -> p h u", u=65)[:, :, 0:64],
                        ps[:, :].rearrange("p (h d) -> p h d", d=64))

                def scores_exp(t, j, kt, slot):
                    d = kt - 4 * j
                    q0 = 128 * d if (causal and d >= 0) else 0
                    ps = sp.tile([128, 1024], f32, tag="s", name=f"ps{t}{j}_{kt}")
                    for half in (0, 1):
                        po = half * 64
                        nc.tensor.matmul(
                            ps[:, half * 512 + q0:(half + 1) * 512],
                            kh[po:po + 64, t, kt * 128:(kt + 1) * 128],
                            qh[po:po + 64, t, j * SC + q0:(j + 1) * SC],
                            start=True, stop=True)
                    pr = probs[:, slot, :, q0:SC]
                    nc.scalar.activation(
                        pr,
                        ps[:, :].rearrange("p (h u) -> p h u", h=2)[:, :, q0:512],
                        EXP)
                    if causal and d >= 0:
                        for half in (0, 1):
                            prh = probs[:, slot, half, q0:SC]
                            nc.vector.tensor_tensor(
                                prh, prh, msk[:, MOFF[d]:MOFF[d] + SC - q0], MUL)

                pts = {}

                def vmm(t, j, kt, slot):
                    d = kt - 4 * j
                    q0 = 128 * d if (causal and d >= 0) else 0
                    nkt_u = 4 * (j + 1) if causal else NKT
                    if kt == 0:
                        pts[(t, j)] = (
                            ptA.tile([65, 512], f32, tag="t0", name=f"pt0_{t}{j}"),
                            ptB.tile([65, 512], f32, tag="t1", name=f"pt1_{t}{j}"))
                    pt = pts[(t, j)]
                    for half in (0, 1):
                        lh = 2 * t + half
                        nc.tensor.matmul(
                            pt[half][:, q0:512],
                            vsb[:, kt, lh * 65:(lh + 1) * 65],
                            probs[:, slot, half, q0:SC],
                            start=(kt == 0), stop=(kt == nkt_u - 1))
                    if kt == nkt_u - 1:
                        for half in (0, 1):
                            lh = 2 * t + half
                            ost = op.tile([65, 512], f32, tag="ost",
                                          name=f"os{half}_{t}{j}")
                            nc.vector.tensor_copy(ost[:, :], pt[half][:, :])
                            nc.gpsimd.dma_start(
                                out=outT_d[lh * 64:(lh + 1) * 64,
                                           j * SC:(j + 1) * SC],
                                in_=ost[0:64, :])
                            nc.gpsimd.dma_start(
                                out=l_d[lh:lh + 1, j * SC:(j + 1) * SC],
                                in_=ost[64:65, :])
                        del pts[(t, j)]

                # ---- static schedule ----
                # startup projections for head-pair 0, queries/keys 0:512;
                # q first (bf16 q path is the long pole; fp8 k loads fast)
                qk_chunk("q", 0, 0)
                qk_chunk("k", 0, 0)

                # HARD emission deadlines (PE queue is in-order, so a v_st or
                # qk chunk emitted after a PE consumer that semaphore-waits on
                # it would deadlock): v_st(s) before instance first_use(s)+VLAG;
                # qk chunks before the first scores matmul that reads them.
                bg_at = defaultdict(list)
                bg_at[3].append(("qk", "q", 0, 1))   # deadline idx 4
                bg_at[5].append(("qk", "k", 0, 1))   # deadline idx 8
                bg_at[7].append(("qk", "q", 0, 2))   # deadline idx 12
                bg_at[9].append(("qk", "k", 0, 2))   # deadline idx 20
                bg_at[11].append(("qk", "q", 0, 3))  # deadline idx 24
                bg_at[13].append(("qk", "k", 0, 3))  # deadline idx 36
                for s in range(NKT):
                    bg_at[4 + 2 * s].append(("v", s))
                for i, wh, m, sc in (
                        (36, "q", 1, 0), (37, "k", 1, 0), (40, "q", 1, 1),
                        (41, "k", 1, 1), (46, "q", 1, 2), (47, "k", 1, 2),
                        (56, "q", 1, 3), (57, "k", 1, 3),
                        (62, "q", 2, 0), (63, "k", 2, 0), (70, "q", 2, 1),
                        (71, "k", 2, 1), (78, "q", 2, 2), (79, "k", 2, 2),
                        (88, "q", 2, 3), (89, "k", 2, 3),
                        (96, "q", 3, 0), (97, "k", 3, 0), (104, "q", 3, 1),
                        (105, "k", 3, 1), (112, "q", 3, 2), (113, "k", 3, 2),
                        (120, "q", 3, 3), (121, "k", 3, 3)):
                    bg_at[i].append(("qk", wh, m, sc))

                instances = []
                for t in range(NT):
                    for j in range(NQC):
                        nkt_u = 4 * (j + 1) if causal else NKT
                        for kt in range(nkt_u):
                            instances.append((t, j, kt))

                vq = deque()

                def drain_vmm(upto):
                    while vq and vq[0][0] <= upto:
                        _, tt, jj, kk, ss = vq.popleft()
                        vmm(tt, jj, kk, ss)

                for idx, (t, j, kt) in enumerate(instances):
                    for item in bg_at.get(idx, ()):
                        if item[0] == "qk":
                            qk_chunk(*item[1:])
                        else:
                            v_st(item[1])
                    drain_vmm(idx - VLAG)
                    slot = idx % NSLOT
                    scores_exp(t, j, kt, slot)
                    vq.append((idx, t, j, kt, slot))
                drain_vmm(10 ** 9)
    _built[causal] = nc
    nc.compile()
    return nc


def _prep_core_inputs(c, q, k, v, Wq, bq, Wk, bk, Wv, bv, sin, cos):
    b, hh = c // 2, c % 2
    hs = slice(hh * DH, (hh + 1) * DH)

    perm = np.empty(DH, np.int64)
    for lh in range(HPC):
        base = (hh * HPC + lh) * HD
        perm[lh * HD:lh * HD + HALF] = base + 2 * np.arange(HALF)
        perm[lh * HD + HALF:(lh + 1) * HD] = base + 2 * np.arange(HALF) + 1

    s = 0.125   # 1/sqrt(HD), folded into the q projection
    S8 = 128.0  # fp8 range scaling for both q/k weights, undone via cs/sn
    wq = np.ascontiguousarray((Wq[perm, :] * (s * S8)).T).astype(BF16)
    wk = np.ascontiguousarray((Wk[perm, :] * S8).T).astype(F8)
    wv = np.ascontiguousarray(Wv[hs, :].T).astype(BF16)

    p32 = np.arange(128) % 32
    cs2 = (cos[:, p32] / S8).T.astype(BF16)
    sgn = np.where((np.arange(128) // 32) % 2 == 0, -1.0, 1.0).astype(np.float32)
    sn2 = (sin[:, p32] * sgn[None, :] / S8).T.astype(BF16)

    kk = np.arange(128)[:, None]
    segs = []
    for d in range(4):
        qq = np.arange(128 * d, 512)[None, :]
        segs.append((128 * d + kk) <= qq)
    msk = np.concatenate(segs, axis=1).astype(BF16)        # [128, 1280]

    bqc = np.ascontiguousarray((bq[perm] * s * S8).reshape(NT, 128).T, np.float32)
    bkc = np.ascontiguousarray((bk[perm] * S8).reshape(NT, 128).T, np.float32)
    bb = np.concatenate([bqc, bkc], axis=1).astype('<f4')  # [128, 8]
    bb16 = np.ascontiguousarray(bb).view('<u2').view(BF16)  # raw halves [128,16]
    msc = np.concatenate([bb16, cs2, sn2], axis=1)

    def pmaj(a):  # [EMB, N] -> [128, NE*N] partition-major slabs
        n = a.shape[1]
        return np.ascontiguousarray(
            a.reshape(NE, 128, n).transpose(1, 0, 2).reshape(128, NE * n))

    def pmaj_m(a):  # [EMB, DH] -> [128, NT*NE*128] m-major slabs
        return np.ascontiguousarray(
            a.reshape(NE, 128, NT, 128).transpose(1, 2, 0, 3).reshape(128, -1))

    def pmaj_sc(a):  # [EMB, SEQ] -> [128, NQC*NE*SC] sc-major slabs
        return np.ascontiguousarray(
            a.reshape(NE, 128, NQC, SC).transpose(1, 2, 0, 3).reshape(128, -1))

    return {
        "xq": pmaj_sc(q[b].T.astype(BF16)),
        "xk": pmaj_sc(k[b].T.astype(F8)),
        "xv": pmaj_sc(v[b].T.astype(BF16)),
        "wq": pmaj_m(wq), "wk": pmaj_m(wk), "wv": pmaj(wv),
        "msc": np.ascontiguousarray(msc), "msk": msk,
    }


def prep_in_maps(q, k, v, Wq, bq, Wk, bk, Wv, bv, sin, cos):
    args = [np.asarray(a, np.float32) for a in (q, k, v, Wq, bq, Wk, bk, Wv, bv, sin, cos)]
    maps = [_prep_core_inputs(c, *args) for c in range(8)]
    return maps, args[8]  # bv needed on host in assemble()


def assemble(results, bv):
    out = np.empty((BATCH, SEQ, EMB), np.float32)
    for c in range(8):
        b, hh = c // 2, c % 2
        outT = np.asarray(results[c]["outT"], np.float32)
        l = np.asarray(results[c]["lsum"], np.float32)
        a = outT.reshape(HPC, HD, SEQ) / l[:, None, :]
        out[b, :, hh * DH:(hh + 1) * DH] = a.reshape(DH, SEQ).T \
            + bv[hh * DH:(hh + 1) * DH][None, :]
    return out


def run(in_maps, causal=True, trace=False, **kw):
    _install_ntff_shim()
    from concourse.bass_utils import run_bass_kernel_spmd
    nc = build(causal)
    return run_bass_kernel_spmd(nc, in_maps, core_ids=list(range(8)), trace=trace, **kw)


def kernel(q, k, v, Wq, bq, Wk, bk, Wv, bv, sin, cos, mask):
    in_maps, bv_f = prep_in_maps(q, k, v, Wq, bq, Wk, bk, Wv, bv, sin, cos)
    r = run(in_maps, causal=bool(mask))
    return assemble(r.results, bv_f)


# revision 60
# speedup vs baseline: 1.0189x; 1.0189x over previous
"""Multi-head causal attention (QKV proj + RoPE + softmax) on 8 TRN2 NeuronCores.

Sharding: batch 4-way x head-group 2-way -> each core handles 1 batch and 8
contiguous heads (512 output channels). No collectives; host gathers slices.

Per-core algorithm (all matmul compute in bf16, fp32 PSUM accumulation):
  - host passes x.T (q/k/v of its batch, transposed to [emb, seq]) and W.T
    shards so every matmul contracts over the partition dim without on-device
    transposes.
  - q/k weights are row-permuted per head into [even dims | odd dims] so RoPE
    becomes: rot = x*cs + swap32(x)*sn, where swap32 is an SBUF partition-block
    swap done by DMA. The per-head dim permutation cancels in q.k dot products.
  - q/k biases are per-partition columns folded into the PSUM eviction
    (tensor_scalar add); the v bias is applied on host: P@(V+b) = P@V + l*b.
  - scores are computed transposed, S_T[k, q] = kh_T.T @ qh_T (K=64
    contraction; the A/B heads of a 128-row tile are emitted adjacently so
    they run concurrently on PE row groups).
  - softmax: exp on ScalarE from PSUM (no max subtraction: |scores| <= ~5 by
    construction); causal mask multiplies on DVE for the diagonal tiles only.
  - attnT[d, q] = sum_kt V_tile[k,d|1].T @ P_T[k, q] -- a ones-column appended
    to V makes row 64 the softmax denominator for free.
  - unnormalized attnT and the denominator row go to HBM via one SBUF staging
    copy; division + final transpose + v-bias happen on host.

Scheduling (the critical part): ScalarE exp is the busiest engine
(~160us); the kernel streams all causal score tiles ("instances") through
ScalarE continuously starting ~9us in.  q/k projections are chunked per
(m-tile, 512-seq chunk) with per-chunk RoPE so head-pair 0 is ready almost
immediately; v-projection chunks and the remaining q/k chunks are emitted as
background PE work on a static pacing map between score matmuls.  exp output
goes to a 12-slot probs ring; the attn@V matmuls lag 8 instances behind exp
so PE never blocks ScalarE.  Diagonal tiles stream only the valid query
columns (saves ~15% on every engine).
"""

import sys
import types
from collections import defaultdict, deque

import numpy as np
import ml_dtypes

BF16 = ml_dtypes.bfloat16
F8 = ml_dtypes.float8_e4m3
SEQ, EMB, NHEADS, BATCH = 2048, 1024, 16, 4
HD, HALF = 64, 32
HPC = 8          # heads per core
DH = 512         # output dims per core
NE = EMB // 128  # 8 contraction tiles
NT = 4           # head-pair (128-row) dout tiles
NKT = SEQ // 128  # 16 key tiles
NQC = SEQ // 512  # 4 query chunks
SC = 512
NSLOT = 12       # probs ring slots
VLAG = 10        # attn@V matmul lag behind exp, in kt-instances
MOFF = [0, 512, 896, 1152]  # packed mask col offsets per diagonal d


def _install_ntff_shim():
    """The image's antenv lacks axon_hooks; synthesize it from trn_agent_boot
    so run_bass_kernel_spmd(trace=True) can profile. Harmless if unused."""
    try:
        import antenv.axon_hooks  # noqa: F401
        return
    except ImportError:
        pass
    try:
        from trn_agent_boot.trn_boot import _ntff_profile_via_ctypes
        import antenv
    except ImportError:
        return
    hook = _ntff_profile_via_ctypes("/opt/axon/libaxon_pjrt.so")
    mod = types.ModuleType("antenv.axon_hooks")
    mod.get_axon_ntff_profile_hook = lambda: hook
    mod.set_axon_ntff_profile_hook = lambda h: None
    sys.modules["antenv.axon_hooks"] = mod
    antenv.axon_hooks = mod


_built = {}


def build(causal=True):
    if causal in _built:
        return _built[causal]
    import concourse.mybir as mybir
    import concourse.tile as tile
    from concourse import bacc

    f32 = mybir.dt.float32
    bf = mybir.dt.bfloat16
    f8 = mybir.dt.float8e4
    DR = mybir.MatmulPerfMode.DoubleRow
    EXP = mybir.ActivationFunctionType.Exp
    MUL = mybir.AluOpType.mult
    ADD = mybir.AluOpType.add

    nc = bacc.Bacc(None, target_bir_lowering=False, debug=False)
    with tile.TileContext(nc) as tc:
        with tc.tile_pool(name="dram", bufs=1, space="DRAM") as dram:
            # x and W arrive partition-major ([128, NE*cols]) so one DMA
            # trigger loads a full [128, NE, colrange] slab
            xq_d = dram.tile([128, NE * SEQ], bf, kind="ExternalInput", name="xq", uniquify=False)
            xk_d = dram.tile([128, NE * SEQ], f8, kind="ExternalInput", name="xk", uniquify=False)
            xv_d = dram.tile([128, NE * SEQ], bf, kind="ExternalInput", name="xv", uniquify=False)
            wq_d = dram.tile([128, NE * DH], bf, kind="ExternalInput", name="wq", uniquify=False)
            wk_d = dram.tile([128, NE * DH], f8, kind="ExternalInput", name="wk", uniquify=False)
            wv_d = dram.tile([128, NE * DH], bf, kind="ExternalInput", name="wv", uniquify=False)
            # bias(8xf32 as 16xbf16 raw) | cs | sn in one small early DMA;
            # mask (single copy, applied per-half) in another
            msc_d = dram.tile([128, 16 + 2 * SEQ], bf, kind="ExternalInput",
                              name="msc", uniquify=False)
            mk_d = dram.tile([128, 1280], bf, kind="ExternalInput",
                             name="msk", uniquify=False)
            # [t, j, half, 65, 512]: rows 0:64 attnT, row 64 denominator
            outc_d = dram.tile([NT * NQC * 2 * 65, SC], f32,
                               kind="ExternalOutput", name="outc", uniquify=False)

            with tc.tile_pool(name="const", bufs=1) as cp, \
                 tc.tile_pool(name="xv", bufs=3) as xvp, \
                 tc.tile_pool(name="rope", bufs=2) as rp, \
                 tc.tile_pool(name="ostage", bufs=3) as op, \
                 tc.tile_pool(name="pp", bufs=2, space="PSUM") as pp, \
                 tc.tile_pool(name="sp", bufs=2, space="PSUM") as sp, \
                 tc.tile_pool(name="tA", bufs=1, space="PSUM") as ptA, \
                 tc.tile_pool(name="tB", bufs=1, space="PSUM") as ptB:

                qh = cp.tile([128, NT, SEQ], bf, name="qh")
                kh = cp.tile([128, NT, SEQ], bf, name="kh")
                vsb = cp.tile([128, NKT, HPC * 65], bf, name="vsb")
                probs = cp.tile([128, NSLOT, 2, SC], bf, name="probs")
                w_sb = {"q": cp.tile([128, NT, NE, 128], bf, name="w_q"),
                        "k": cp.tile([128, NT, NE, 128], f8, name="w_k"),
                        "v": cp.tile([128, NE, DH], bf, name="w_v")}
                msc = cp.tile([128, 16 + 2 * SEQ], bf, name="msc")
                bb = msc[:, 0:16].bitcast(f32)          # [128, 8] f32
                b_sb = {"q": bb[:, 0:NT], "k": bb[:, NT:2 * NT]}
                cs = msc[:, 16:16 + SEQ]
                sn = msc[:, 16 + SEQ:16 + 2 * SEQ]
                msk = cp.tile([128, 1280], bf, name="mskt")
                scr = cp.tile([128, 16], bf, name="scr")
                scrW = cp.tile([128, 64], bf, name="scrW")
                scrX = cp.tile([128, 512], bf, name="scrX")

                # ---- warmup: ACT table preload + PE HAM spin-up (independent
                # scratch tiles so the MMs don't serialize behind ScalarE)
                nc.vector.memset(scr[:, :], 0.0)
                nc.vector.memset(scrW[:, :], 0.0)
                nc.vector.memset(scrX[:, :], 0.0)
                nc.scalar.activation(scr[:, 0:8], scr[:, 8:16], EXP)
                wps = pp.tile([128, SC], f32, tag="p", name="warm")
                for r in range(10):
                    nc.tensor.matmul(wps[0:64, :], scrW[:, :], scrX[:, :],
                                     start=True, stop=True)

                # ---- input loads: ONE queue (sync), strict priority order so
                # DMA-engine bandwidth serves the critical path first. x
                # tensors arrive sc-major, W m-major: every load is one
                # fully-contiguous slab. ----
                xq_v = xq_d[:, :].rearrange("p (c e s) -> p c e s", c=NQC, e=NE)
                xk_v = xk_d[:, :].rearrange("p (c e s) -> p c e s", c=NQC, e=NE)
                xv_v = xv_d[:, :].rearrange("p (c e s) -> p c e s", c=NQC, e=NE)
                wq_v = wq_d[:, :].rearrange("p (m e c) -> p m e c", m=NT, e=NE)
                wk_v = wk_d[:, :].rearrange("p (m e c) -> p m e c", m=NT, e=NE)
                wv_v = wv_d[:, :].rearrange("p (e c) -> p e c", e=NE)

                xq_sb = cp.tile([128, NQC, NE, SC], bf, name="xq_sb")
                xk_sb = cp.tile([128, NQC, NE, SC], f8, name="xk_sb")
                xv_sc = {}

                def load_x_sc(dst, srcv, sc):
                    nc.sync.dma_start(out=dst[:, sc, :, :], in_=srcv[:, sc, :, :])

                def load_xv_sc(sc):
                    t = xvp.tile([128, NE, SC], bf, tag="xv", name=f"xv{sc}")
                    nc.sync.dma_start(out=t[:, :, :], in_=xv_v[:, sc, :, :])
                    xv_sc[sc] = t

                nc.sync.dma_start(out=w_sb["q"][:, 0, :, :], in_=wq_v[:, 0, :, :])
                load_x_sc(xq_sb, xq_v, 0)
                nc.sync.dma_start(out=w_sb["k"][:, 0, :, :], in_=wk_v[:, 0, :, :])
                load_x_sc(xk_sb, xk_v, 0)
                nc.sync.dma_start(out=msc[:, :], in_=msc_d[:, :])
                nc.sync.dma_start(out=msk[:, :], in_=mk_d[:, :])
                load_x_sc(xq_sb, xq_v, 1)
                load_x_sc(xk_sb, xk_v, 1)
                nc.sync.dma_start(out=w_sb["v"][:, :, :], in_=wv_v[:, :, :])
                load_xv_sc(0)
                load_x_sc(xq_sb, xq_v, 2)
                load_x_sc(xk_sb, xk_v, 2)
                load_xv_sc(1)
                load_x_sc(xq_sb, xq_v, 3)
                load_x_sc(xk_sb, xk_v, 3)
                nc.sync.dma_start(out=w_sb["q"][:, 1:NT, :, :], in_=wq_v[:, 1:NT, :, :])
                nc.sync.dma_start(out=w_sb["k"][:, 1:NT, :, :], in_=wk_v[:, 1:NT, :, :])
                load_xv_sc(2)
                load_xv_sc(3)
                # only the ones-columns (col 64 of each 65-block) need init;
                # the v evictions overwrite the 64 data columns of every block
                nc.vector.memset(
                    vsb[:, :, :].rearrange("p k (h u) -> p k h u", u=65)[:, :, :, 64:65],
                    1.0)

                # ---- emitters ----
                def qk_chunk(which, m, sc):
                    # k: fp8 DoubleRow, 2 e-tiles per matmul (virtual K=256).
                    # q: bf16 col-split pairs (fp8 on both q and k pushes the
                    # softmax error past the gate; one side is safe).
                    x_sb = xq_sb if which == "q" else xk_sb
                    dst = qh if which == "q" else kh
                    ps = pp.tile([128, SC], f32, tag="p", name=f"pp{which}{m}{sc}")
                    if which == "k":
                        for ep in range(NE // 2):
                            nc.tensor.matmul(ps[:, :],
                                             w_sb["k"][:, m, 2 * ep:2 * ep + 2, :],
                                             x_sb[:, sc, 2 * ep:2 * ep + 2, :],
                                             start=(ep == 0),
                                             stop=(ep == NE // 2 - 1),
                                             perf_mode=DR)
                    else:
                        for e in range(NE):
                            nc.tensor.matmul(ps[0:64, :],
                                             w_sb["q"][:, m, e, 0:64],
                                             x_sb[:, sc, e, :],
                                             start=(e == 0), stop=(e == NE - 1))
                            nc.tensor.matmul(ps[64:128, :],
                                             w_sb["q"][:, m, e, 64:128],
                                             x_sb[:, sc, e, :],
                                             start=(e == 0), stop=(e == NE - 1))
                    if m == 0:
                        # per-chunk rope so head-pair 0 streams immediately
                        tmp = rp.tile([128, SC], bf, tag="tmp", bufs=3,
                                      name=f"t{which}{m}{sc}")
                        nc.vector.tensor_scalar_add(tmp[:, :], ps[:, :],
                                                    b_sb[which][:, m:m + 1])
                        tsw = rp.tile([128, SC], bf, tag="tsw", name=f"w{which}{m}{sc}")
                        for blk in range(4):
                            s = blk ^ 1
                            nc.gpsimd.dma_start(out=tsw[blk * 32:(blk + 1) * 32, :],
                                                in_=tmp[s * 32:(s + 1) * 32, :])
                        m2 = rp.tile([128, SC], bf, tag="m2", name=f"m{which}{m}{sc}")
                        dsl = dst[:, m, sc * SC:(sc + 1) * SC]
                        nc.vector.tensor_tensor(dsl, tmp[:, :],
                                                cs[:, sc * SC:(sc + 1) * SC], MUL)
                        nc.vector.tensor_tensor(m2[:, :], tsw[:, :],
                                                sn[:, sc * SC:(sc + 1) * SC], MUL)
                        nc.vector.tensor_tensor(dsl, dsl, m2[:, :], ADD)
                    else:
                        # evict pre-rope values into the dst row; one in-place
                        # rope over the whole row after the last chunk
                        nc.vector.tensor_scalar_add(
                            dst[:, m, sc * SC:(sc + 1) * SC], ps[:, :],
                            b_sb[which][:, m:m + 1])
                        if sc == 3:
                            row = dst[:, m, :]
                            tswm = rp.tile([128, SEQ], bf, tag="tswm",
                                           name=f"wm{which}{m}")
                            for blk in range(4):
                                s = blk ^ 1
                                nc.sync.dma_start(
                                    out=tswm[blk * 32:(blk + 1) * 32, :],
                                    in_=dst[s * 32:(s + 1) * 32, m, :])
                            m2m = rp.tile([128, SEQ], bf, tag="m2m",
                                          name=f"mm{which}{m}")
                            nc.vector.tensor_tensor(row, row, cs[:, :], MUL)
                            nc.vector.tensor_tensor(m2m[:, :], tswm[:, :],
                                                    sn[:, :], MUL)
                            nc.vector.tensor_tensor(row, row, m2m[:, :], ADD)

                def v_st(st):
                    sc, o = st // 4, (st % 4) * 128
                    xt = xv_sc[sc]
                    ps = pp.tile([128, SC], f32, tag="p", name=f"ppv{st}")
                    for e in range(NE):
                        nc.tensor.matmul(ps[0:64, :], xt[:, e, o:o + 64],
                                         w_sb["v"][:, e, :],
                                         start=(e == 0), stop=(e == NE - 1))
                        nc.tensor.matmul(ps[64:128, :], xt[:, e, o + 64:o + 128],
                                         w_sb["v"][:, e, :],
                                         start=(e == 0), stop=(e == NE - 1))
                    nc.vector.tensor_copy(
                        vsb[:, st, :].rearrange("p (h u) -> p h u", u=65)[:, :, 0:64],
                        ps[:, :].rearrange("p (h d) -> p h d", d=64))

                def scores_exp(t, j, kt, slot):
                    d = kt - 4 * j
                    q0 = 128 * d if (causal and d >= 0) else 0
                    ps = sp.tile([128, 1024], f32, tag="s", name=f"ps{t}{j}_{kt}")
                    for half in (0, 1):
                        po = half * 64
                        nc.tensor.matmul(
                            ps[:, half * 512 + q0:(half + 1) * 512],
                            kh[po:po + 64, t, kt * 128:(kt + 1) * 128],
                            qh[po:po + 64, t, j * SC + q0:(j + 1) * SC],
                            start=True, stop=True)
                    pr = probs[:, slot, :, q0:SC]
                    nc.scalar.activation(
                        pr,
                        ps[:, :].rearrange("p (h u) -> p h u", h=2)[:, :, q0:512],
                        EXP)
                    if causal and d >= 0:
                        for half in (0, 1):
                            prh = probs[:, slot, half, q0:SC]
                            nc.vector.tensor_tensor(
                                prh, prh, msk[:, MOFF[d]:MOFF[d] + SC - q0], MUL)

                pts = {}

                def vmm(t, j, kt, slot):
                    d = kt - 4 * j
                    q0 = 128 * d if (causal and d >= 0) else 0
                    nkt_u = 4 * (j + 1) if causal else NKT
                    if kt == 0:
                        pts[(t, j)] = (
                            ptA.tile([65, 512], f32, tag="t0", name=f"pt0_{t}{j}"),
                            ptB.tile([65, 512], f32, tag="t1", name=f"pt1_{t}{j}"))
                    pt = pts[(t, j)]
                    for half in (0, 1):
                        lh = 2 * t + half
                        nc.tensor.matmul(
                            pt[half][:, q0:512],
                            vsb[:, kt, lh * 65:(lh + 1) * 65],
                            probs[:, slot, half, q0:SC],
                            start=(kt == 0), stop=(kt == nkt_u - 1))
                    if kt == nkt_u - 1:
                        for half in (0, 1):
                            ost = op.tile([65, 512], f32, tag="ost",
                                          name=f"os{half}_{t}{j}")
                            nc.vector.tensor_copy(ost[:, :], pt[half][:, :])
                            r0 = ((t * NQC + j) * 2 + half) * 65
                            nc.gpsimd.dma_start(out=outc_d[r0:r0 + 65, :],
                                                in_=ost[:, :])
                        del pts[(t, j)]

                # ---- static schedule ----
                # startup projections for head-pair 0, queries/keys 0:512;
                # q first (bf16 q path is the long pole; fp8 k loads fast)
                qk_chunk("q", 0, 0)
                qk_chunk("k", 0, 0)

                # HARD emission deadlines (PE queue is in-order, so a v_st or
                # qk chunk emitted after a PE consumer that semaphore-waits on
                # it would deadlock): v_st(s) before instance first_use(s)+VLAG;
                # qk chunks before the first scores matmul that reads them.
                bg_at = defaultdict(list)
                bg_at[3].append(("qk", "q", 0, 1))   # deadline idx 4
                bg_at[5].append(("qk", "k", 0, 1))   # deadline idx 8
                bg_at[7].append(("qk", "q", 0, 2))   # deadline idx 12
                bg_at[9].append(("qk", "k", 0, 2))   # deadline idx 20
                bg_at[11].append(("qk", "q", 0, 3))  # deadline idx 24
                bg_at[13].append(("qk", "k", 0, 3))  # deadline idx 36
                for s in range(NKT):
                    bg_at[4 + 2 * s].append(("v", s))
                for i, wh, m, sc in (
                        (25, "q", 1, 0), (26, "k", 1, 0), (27, "q", 1, 1),
                        (28, "k", 1, 1), (29, "q", 1, 2), (30, "k", 1, 2),
                        (31, "q", 1, 3), (32, "k", 1, 3),
                        (44, "q", 2, 0), (46, "k", 2, 0), (48, "q", 2, 1),
                        (50, "k", 2, 1), (52, "q", 2, 2), (54, "k", 2, 2),
                        (56, "q", 2, 3), (58, "k", 2, 3),
                        (84, "q", 3, 0), (86, "k", 3, 0), (88, "q", 3, 1),
                        (90, "k", 3, 1), (92, "q", 3, 2), (94, "k", 3, 2),
                        (96, "q", 3, 3), (98, "k", 3, 3)):
                    bg_at[i].append(("qk", wh, m, sc))

                instances = []
                for t in range(NT):
                    for j in range(NQC):
                        nkt_u = 4 * (j + 1) if causal else NKT
                        for kt in range(nkt_u):
                            instances.append((t, j, kt))

                vq = deque()

                def drain_vmm(upto):
                    while vq and vq[0][0] <= upto:
                        _, tt, jj, kk, ss = vq.popleft()
                        vmm(tt, jj, kk, ss)

                for idx, (t, j, kt) in enumerate(instances):
                    for item in bg_at.get(idx, ()):
                        if item[0] == "qk":
                            qk_chunk(*item[1:])
                        else:
                            v_st(item[1])
                    drain_vmm(idx - VLAG)
                    slot = idx % NSLOT
                    scores_exp(t, j, kt, slot)
                    vq.append((idx, t, j, kt, slot))
                drain_vmm(10 ** 9)
    _built[causal] = nc
    nc.compile()
    return nc


def _prep_core_inputs(c, q, k, v, Wq, bq, Wk, bk, Wv, bv, sin, cos):
    b, hh = c // 2, c % 2
    hs = slice(hh * DH, (hh + 1) * DH)

    perm = np.empty(DH, np.int64)
    for lh in range(HPC):
        base = (hh * HPC + lh) * HD
        perm[lh * HD:lh * HD + HALF] = base + 2 * np.arange(HALF)
        perm[lh * HD + HALF:(lh + 1) * HD] = base + 2 * np.arange(HALF) + 1

    s = 0.125   # 1/sqrt(HD), folded into the q projection
    S8 = 128.0  # fp8 range scaling for both q/k weights, undone via cs/sn
    wq = np.ascontiguousarray((Wq[perm, :] * (s * S8)).T).astype(BF16)
    wk = np.ascontiguousarray((Wk[perm, :] * S8).T).astype(F8)
    wv = np.ascontiguousarray(Wv[hs, :].T).astype(BF16)

    p32 = np.arange(128) % 32
    cs2 = (cos[:, p32] / S8).T.astype(BF16)
    sgn = np.where((np.arange(128) // 32) % 2 == 0, -1.0, 1.0).astype(np.float32)
    sn2 = (sin[:, p32] * sgn[None, :] / S8).T.astype(BF16)

    kk = np.arange(128)[:, None]
    segs = []
    for d in range(4):
        qq = np.arange(128 * d, 512)[None, :]
        segs.append((128 * d + kk) <= qq)
    msk = np.concatenate(segs, axis=1).astype(BF16)        # [128, 1280]

    bqc = np.ascontiguousarray((bq[perm] * s * S8).reshape(NT, 128).T, np.float32)
    bkc = np.ascontiguousarray((bk[perm] * S8).reshape(NT, 128).T, np.float32)
    bb = np.concatenate([bqc, bkc], axis=1).astype('<f4')  # [128, 8]
    bb16 = np.ascontiguousarray(bb).view('<u2').view(BF16)  # raw halves [128,16]
    msc = np.concatenate([bb16, cs2, sn2], axis=1)

    def pmaj(a):  # [EMB, N] -> [128, NE*N] partition-major slabs
        n = a.shape[1]
        return np.ascontiguousarray(
            a.reshape(NE, 128, n).transpose(1, 0, 2).reshape(128, NE * n))

    def pmaj_m(a):  # [EMB, DH] -> [128, NT*NE*128] m-major slabs
        return np.ascontiguousarray(
            a.reshape(NE, 128, NT, 128).transpose(1, 2, 0, 3).reshape(128, -1))

    def pmaj_sc(a):  # [EMB, SEQ] -> [128, NQC*NE*SC] sc-major slabs
        return np.ascontiguousarray(
            a.reshape(NE, 128, NQC, SC).transpose(1, 2, 0, 3).reshape(128, -1))

    return {
        "xq": pmaj_sc(q[b].T.astype(BF16)),
        "xk": pmaj_sc(k[b].T.astype(F8)),
        "xv": pmaj_sc(v[b].T.astype(BF16)),
        "wq": pmaj_m(wq), "wk": pmaj_m(wk), "wv": pmaj(wv),
        "msc": np.ascontiguousarray(msc), "msk": msk,
    }


def prep_in_maps(q, k, v, Wq, bq, Wk, bk, Wv, bv, sin, cos):
    args = [np.asarray(a, np.float32) for a in (q, k, v, Wq, bq, Wk, bk, Wv, bv, sin, cos)]
    maps = [_prep_core_inputs(c, *args) for c in range(8)]
    return maps, args[8]  # bv needed on host in assemble()


def assemble(results, bv):
    out = np.empty((BATCH, SEQ, EMB), np.float32)
    for c in range(8):
        b, hh = c // 2, c % 2
        oc = np.asarray(results[c]["outc"], np.float32).reshape(NT, NQC, 2, 65, SC)
        an = oc[:, :, :, 0:64, :] / oc[:, :, :, 64:65, :]   # [t, j, half, 64, q]
        an = an.transpose(0, 2, 3, 1, 4).reshape(DH, SEQ)   # [(t,half,d), (j,q)]
        out[b, :, hh * DH:(hh + 1) * DH] = an.T \
            + bv[hh * DH:(hh + 1) * DH][None, :]
    return out


def run(in_maps, causal=True, trace=False, **kw):
    _install_ntff_shim()
    from concourse.bass_utils import run_bass_kernel_spmd
    nc = build(causal)
    return run_bass_kernel_spmd(nc, in_maps, core_ids=list(range(8)), trace=trace, **kw)


def kernel(q, k, v, Wq, bq, Wk, bk, Wv, bv, sin, cos, mask):
    in_maps, bv_f = prep_in_maps(q, k, v, Wq, bq, Wk, bk, Wv, bv, sin, cos)
    r = run(in_maps, causal=bool(mask))
    return assemble(r.results, bv_f)


# revision 63
# speedup vs baseline: 1.2139x; 1.1914x over previous
"""Multi-head causal attention (QKV proj + RoPE + softmax) on 8 TRN2 NeuronCores.

Sharding: batch 4-way x head-group 2-way -> each core handles 1 batch and 8
contiguous heads (512 output channels). No collectives; host gathers slices.

Per-core algorithm (all matmul compute in bf16, fp32 PSUM accumulation):
  - host passes x.T (q/k/v of its batch, transposed to [emb, seq]) and W.T
    shards so every matmul contracts over the partition dim without on-device
    transposes.
  - q/k weights are row-permuted per head into [even dims | odd dims] so RoPE
    becomes: rot = x*cs + swap32(x)*sn, where swap32 is an SBUF partition-block
    swap done by DMA. The per-head dim permutation cancels in q.k dot products.
  - q/k biases are per-partition columns folded into the PSUM eviction
    (tensor_scalar add); the v bias is applied on host: P@(V+b) = P@V + l*b.
  - scores are computed transposed, S_T[k, q] = kh_T.T @ qh_T (K=64
    contraction; the A/B heads of a 128-row tile are emitted adjacently so
    they run concurrently on PE row groups).
  - softmax: exp on ScalarE from PSUM (no max subtraction: |scores| <= ~5 by
    construction); causal mask multiplies on DVE for the diagonal tiles only.
  - attnT[d, q] = sum_kt V_tile[k,d|1].T @ P_T[k, q] -- a ones-column appended
    to V makes row 64 the softmax denominator for free.
  - unnormalized attnT and the denominator row go to HBM via one SBUF staging
    copy; division + final transpose + v-bias happen on host.

Scheduling (the critical part): ScalarE exp is the busiest engine
(~160us); the kernel streams all causal score tiles ("instances") through
ScalarE continuously starting ~9us in.  q/k projections are chunked per
(m-tile, 512-seq chunk) with per-chunk RoPE so head-pair 0 is ready almost
immediately; v-projection chunks and the remaining q/k chunks are emitted as
background PE work on a static pacing map between score matmuls.  exp output
goes to a 12-slot probs ring; the attn@V matmuls lag 8 instances behind exp
so PE never blocks ScalarE.  Diagonal tiles stream only the valid query
columns (saves ~15% on every engine).
"""

import sys
import types
from collections import defaultdict, deque

import numpy as np
import ml_dtypes

BF16 = ml_dtypes.bfloat16
F8 = ml_dtypes.float8_e4m3
SEQ, EMB, NHEADS, BATCH = 2048, 1024, 16, 4
HD, HALF = 64, 32
HPC = 8          # heads per core
DH = 512         # output dims per core
NE = EMB // 128  # 8 contraction tiles
NT = 4           # head-pair (128-row) dout tiles
NKT = SEQ // 128  # 16 key tiles
NQC = SEQ // 512  # 4 query chunks
SC = 512
NSLOT = 12       # probs ring slots
VLAG = 10        # attn@V matmul lag behind exp, in kt-instances
MOFF = [0, 512, 896, 1152]  # packed mask col offsets per diagonal d


def _install_ntff_shim():
    """The image's antenv lacks axon_hooks; synthesize it from trn_agent_boot
    so run_bass_kernel_spmd(trace=True) can profile. Harmless if unused."""
    try:
        import antenv.axon_hooks  # noqa: F401
        return
    except ImportError:
        pass
    try:
        from trn_agent_boot.trn_boot import _ntff_profile_via_ctypes
        import antenv
    except ImportError:
        return
    hook = _ntff_profile_via_ctypes("/opt/axon/libaxon_pjrt.so")
    mod = types.ModuleType("antenv.axon_hooks")
    mod.get_axon_ntff_profile_hook = lambda: hook
    mod.set_axon_ntff_profile_hook = lambda h: None
    sys.modules["antenv.axon_hooks"] = mod
    antenv.axon_hooks = mod


_built = {}


def build(causal=True):
    if causal in _built:
        return _built[causal]
    import concourse.mybir as mybir
    import concourse.tile as tile
    from concourse import bacc

    f32 = mybir.dt.float32
    bf = mybir.dt.bfloat16
    f8 = mybir.dt.float8e4
    DR = mybir.MatmulPerfMode.DoubleRow
    EXP = mybir.ActivationFunctionType.Exp
    MUL = mybir.AluOpType.mult
    ADD = mybir.AluOpType.add

    nc = bacc.Bacc(None, target_bir_lowering=False, debug=False)
    with tile.TileContext(nc) as tc:
        with tc.tile_pool(name="dram", bufs=1, space="DRAM") as dram:
            # x and W arrive partition-major ([128, NE*cols]) so one DMA
            # trigger loads a full [128, NE, colrange] slab
            xq_d = dram.tile([128, NE * SEQ], bf, kind="ExternalInput", name="xq", uniquify=False)
            xk_d = dram.tile([128, NE * SEQ], f8, kind="ExternalInput", name="xk", uniquify=False)
            xv_d = dram.tile([128, NE * SEQ], bf, kind="ExternalInput", name="xv", uniquify=False)
            wq_d = dram.tile([128, NE * DH], bf, kind="ExternalInput", name="wq", uniquify=False)
            wk_d = dram.tile([128, NE * DH], f8, kind="ExternalInput", name="wk", uniquify=False)
            wv_d = dram.tile([128, NE * DH], bf, kind="ExternalInput", name="wv", uniquify=False)
            # bias(8xf32 as 16xbf16 raw) | cs | sn in one small early DMA;
            # mask (single copy, applied per-half) in another
            msc_d = dram.tile([128, 16 + 2 * SEQ], bf, kind="ExternalInput",
                              name="msc", uniquify=False)
            mk_d = dram.tile([128, 1280], bf, kind="ExternalInput",
                             name="msk", uniquify=False)
            # [t, j, half, 65, 512]: rows 0:64 attnT, row 64 denominator
            outc_d = dram.tile([NT * NQC * 2 * 65, SC], f32,
                               kind="ExternalOutput", name="outc", uniquify=False)

            with tc.tile_pool(name="const", bufs=1) as cp, \
                 tc.tile_pool(name="xv", bufs=3) as xvp, \
                 tc.tile_pool(name="rope", bufs=2) as rp, \
                 tc.tile_pool(name="ostage", bufs=3) as op, \
                 tc.tile_pool(name="pp", bufs=2, space="PSUM") as pp, \
                 tc.tile_pool(name="sp", bufs=2, space="PSUM") as sp, \
                 tc.tile_pool(name="tA", bufs=1, space="PSUM") as ptA, \
                 tc.tile_pool(name="tB", bufs=1, space="PSUM") as ptB:

                qh = cp.tile([128, NT, SEQ], bf, name="qh")
                kh = cp.tile([128, NT, SEQ], bf, name="kh")
                vsb = cp.tile([128, NKT, HPC * 65], bf, name="vsb")
                probs = cp.tile([128, NSLOT, 2, SC], bf, name="probs")
                w_sb = {"q": cp.tile([128, NT, NE, 128], bf, name="w_q"),
                        "k": cp.tile([128, NT, NE, 128], f8, name="w_k"),
                        "v": cp.tile([128, NE, DH], bf, name="w_v")}
                msc = cp.tile([128, 16 + 2 * SEQ], bf, name="msc")
                bb = msc[:, 0:16].bitcast(f32)          # [128, 8] f32
                b_sb = {"q": bb[:, 0:NT], "k": bb[:, NT:2 * NT]}
                cs = msc[:, 16:16 + SEQ]
                sn = msc[:, 16 + SEQ:16 + 2 * SEQ]
                msk = cp.tile([128, 1280], bf, name="mskt")
                scr = cp.tile([128, 16], bf, name="scr")
                scrW = cp.tile([128, 64], bf, name="scrW")
                scrX = cp.tile([128, 512], bf, name="scrX")

                # ---- warmup: ACT table preload + PE HAM spin-up (independent
                # scratch tiles so the MMs don't serialize behind ScalarE)
                nc.vector.memset(scr[:, :], 0.0)
                nc.vector.memset(scrW[:, :], 0.0)
                nc.vector.memset(scrX[:, :], 0.0)
                nc.scalar.activation(scr[:, 0:8], scr[:, 8:16], EXP)
                wps = pp.tile([128, SC], f32, tag="p", name="warm")
                for r in range(10):
                    nc.tensor.matmul(wps[0:64, :], scrW[:, :], scrX[:, :],
                                     start=True, stop=True)

                # ---- input loads: ONE queue (sync), strict priority order so
                # DMA-engine bandwidth serves the critical path first. x
                # tensors arrive sc-major, W m-major: every load is one
                # fully-contiguous slab. ----
                xq_v = xq_d[:, :].rearrange("p (c e s) -> p c e s", c=NQC, e=NE)
                xk_v = xk_d[:, :].rearrange("p (c e s) -> p c e s", c=NQC, e=NE)
                xv_v = xv_d[:, :].rearrange("p (c e s) -> p c e s", c=NQC, e=NE)
                wq_v = wq_d[:, :].rearrange("p (m e c) -> p m e c", m=NT, e=NE)
                wk_v = wk_d[:, :].rearrange("p (m e c) -> p m e c", m=NT, e=NE)
                wv_v = wv_d[:, :].rearrange("p (e c) -> p e c", e=NE)

                xq_sb = cp.tile([128, NQC, NE, SC], bf, name="xq_sb")
                xk_sb = cp.tile([128, NQC, NE, SC], f8, name="xk_sb")
                xv_sc = {}

                def load_x_sc(dst, srcv, sc):
                    nc.sync.dma_start(out=dst[:, sc, :, :], in_=srcv[:, sc, :, :])

                def load_xv_sc(sc):
                    t = xvp.tile([128, NE, SC], bf, tag="xv", name=f"xv{sc}")
                    nc.sync.dma_start(out=t[:, :, :], in_=xv_v[:, sc, :, :])
                    xv_sc[sc] = t

                nc.sync.dma_start(out=w_sb["q"][:, 0, :, :], in_=wq_v[:, 0, :, :])
                load_x_sc(xq_sb, xq_v, 0)
                nc.sync.dma_start(out=w_sb["k"][:, 0, :, :], in_=wk_v[:, 0, :, :])
                load_x_sc(xk_sb, xk_v, 0)
                nc.sync.dma_start(out=msc[:, :], in_=msc_d[:, :])
                load_x_sc(xq_sb, xq_v, 1)
                load_x_sc(xk_sb, xk_v, 1)
                nc.sync.dma_start(out=msk[:, :], in_=mk_d[:, :])
                nc.sync.dma_start(out=w_sb["v"][:, :, :], in_=wv_v[:, :, :])
                load_xv_sc(0)
                load_x_sc(xq_sb, xq_v, 2)
                load_x_sc(xk_sb, xk_v, 2)
                load_xv_sc(1)
                nc.sync.dma_start(out=w_sb["q"][:, 1:NT, :, :], in_=wq_v[:, 1:NT, :, :])
                nc.sync.dma_start(out=w_sb["k"][:, 1:NT, :, :], in_=wk_v[:, 1:NT, :, :])
                load_x_sc(xq_sb, xq_v, 3)
                load_x_sc(xk_sb, xk_v, 3)
                load_xv_sc(2)
                load_xv_sc(3)
                # only the ones-columns (col 64 of each 65-block) need init;
                # the v evictions overwrite the 64 data columns of every block
                nc.vector.memset(
                    vsb[:, :, :].rearrange("p k (h u) -> p k h u", u=65)[:, :, :, 64:65],
                    1.0)

                # ---- emitters ----
                def qk_chunk(which, m, sc):
                    # k: fp8 DoubleRow, 2 e-tiles per matmul (virtual K=256).
                    # q: bf16 col-split pairs (fp8 on both q and k pushes the
                    # softmax error past the gate; one side is safe).
                    x_sb = xq_sb if which == "q" else xk_sb
                    dst = qh if which == "q" else kh
                    ps = pp.tile([128, SC], f32, tag="p", name=f"pp{which}{m}{sc}")
                    if which == "k":
                        for ep in range(NE // 2):
                            nc.tensor.matmul(ps[:, :],
                                             w_sb["k"][:, m, 2 * ep:2 * ep + 2, :],
                                             x_sb[:, sc, 2 * ep:2 * ep + 2, :],
                                             start=(ep == 0),
                                             stop=(ep == NE // 2 - 1),
                                             perf_mode=DR)
                    else:
                        for e in range(NE):
                            nc.tensor.matmul(ps[0:64, :],
                                             w_sb["q"][:, m, e, 0:64],
                                             x_sb[:, sc, e, :],
                                             start=(e == 0), stop=(e == NE - 1))
                            nc.tensor.matmul(ps[64:128, :],
                                             w_sb["q"][:, m, e, 64:128],
                                             x_sb[:, sc, e, :],
                                             start=(e == 0), stop=(e == NE - 1))
                    if m == 0:
                        # per-chunk rope so head-pair 0 streams immediately
                        tmp = rp.tile([128, SC], bf, tag="tmp", bufs=3,
                                      name=f"t{which}{m}{sc}")
                        nc.vector.tensor_scalar_add(tmp[:, :], ps[:, :],
                                                    b_sb[which][:, m:m + 1])
                        tsw = rp.tile([128, SC], bf, tag="tsw", name=f"w{which}{m}{sc}")
                        for blk in range(4):
                            s = blk ^ 1
                            nc.gpsimd.dma_start(out=tsw[blk * 32:(blk + 1) * 32, :],
                                                in_=tmp[s * 32:(s + 1) * 32, :])
                        m2 = rp.tile([128, SC], bf, tag="m2", name=f"m{which}{m}{sc}")
                        dsl = dst[:, m, sc * SC:(sc + 1) * SC]
                        nc.vector.tensor_tensor(dsl, tmp[:, :],
                                                cs[:, sc * SC:(sc + 1) * SC], MUL)
                        nc.vector.tensor_tensor(m2[:, :], tsw[:, :],
                                                sn[:, sc * SC:(sc + 1) * SC], MUL)
                        nc.vector.tensor_tensor(dsl, dsl, m2[:, :], ADD)
                    else:
                        # evict pre-rope values into the dst row; one in-place
                        # rope over the whole row after the last chunk
                        nc.vector.tensor_scalar_add(
                            dst[:, m, sc * SC:(sc + 1) * SC], ps[:, :],
                            b_sb[which][:, m:m + 1])
                        if sc == 3:
                            row = dst[:, m, :]
                            tswm = rp.tile([128, SEQ], bf, tag="tswm",
                                           name=f"wm{which}{m}")
                            for blk in range(4):
                                s = blk ^ 1
                                nc.sync.dma_start(
                                    out=tswm[blk * 32:(blk + 1) * 32, :],
                                    in_=dst[s * 32:(s + 1) * 32, m, :])
                            m2m = rp.tile([128, SEQ], bf, tag="m2m",
                                          name=f"mm{which}{m}")
                            nc.vector.tensor_tensor(row, row, cs[:, :], MUL)
                            nc.vector.tensor_tensor(m2m[:, :], tswm[:, :],
                                                    sn[:, :], MUL)
                            nc.vector.tensor_tensor(row, row, m2m[:, :], ADD)

                def v_st(st):
                    sc, o = st // 4, (st % 4) * 128
                    xt = xv_sc[sc]
                    ps = pp.tile([128, SC], f32, tag="p", name=f"ppv{st}")
                    for e in range(NE):
                        nc.tensor.matmul(ps[0:64, :], xt[:, e, o:o + 64],
                                         w_sb["v"][:, e, :],
                                         start=(e == 0), stop=(e == NE - 1))
                        nc.tensor.matmul(ps[64:128, :], xt[:, e, o + 64:o + 128],
                                         w_sb["v"][:, e, :],
                                         start=(e == 0), stop=(e == NE - 1))
                    nc.vector.tensor_copy(
                        vsb[:, st, :].rearrange("p (h u) -> p h u", u=65)[:, :, 0:64],
                        ps[:, :].rearrange("p (h d) -> p h d", d=64))

                def scores_exp(t, j, kt, slot):
                    d = kt - 4 * j
                    q0 = 128 * d if (causal and d >= 0) else 0
                    ps = sp.tile([128, 1024], f32, tag="s", name=f"ps{t}{j}_{kt}")
                    for half in (0, 1):
                        po = half * 64
                        nc.tensor.matmul(
                            ps[:, half * 512 + q0:(half + 1) * 512],
                            kh[po:po + 64, t, kt * 128:(kt + 1) * 128],
                            qh[po:po + 64, t, j * SC + q0:(j + 1) * SC],
                            start=True, stop=True)
                    pr = probs[:, slot, :, q0:SC]
                    nc.scalar.activation(
                        pr,
                        ps[:, :].rearrange("p (h u) -> p h u", h=2)[:, :, q0:512],
                        EXP)
                    if causal and d >= 0:
                        for half in (0, 1):
                            prh = probs[:, slot, half, q0:SC]
                            nc.vector.tensor_tensor(
                                prh, prh, msk[:, MOFF[d]:MOFF[d] + SC - q0], MUL)

                pts = {}

                def vmm(t, j, kt, slot):
                    d = kt - 4 * j
                    q0 = 128 * d if (causal and d >= 0) else 0
                    nkt_u = 4 * (j + 1) if causal else NKT
                    if kt == 0:
                        pts[(t, j)] = (
                            ptA.tile([65, 512], f32, tag="t0", name=f"pt0_{t}{j}"),
                            ptB.tile([65, 512], f32, tag="t1", name=f"pt1_{t}{j}"))
                    pt = pts[(t, j)]
                    for half in (0, 1):
                        lh = 2 * t + half
                        nc.tensor.matmul(
                            pt[half][:, q0:512],
                            vsb[:, kt, lh * 65:(lh + 1) * 65],
                            probs[:, slot, half, q0:SC],
                            start=(kt == 0), stop=(kt == nkt_u - 1))
                    if kt == nkt_u - 1:
                        for half in (0, 1):
                            ost = op.tile([65, 512], f32, tag="ost",
                                          name=f"os{half}_{t}{j}")
                            nc.vector.tensor_copy(ost[:, :], pt[half][:, :])
                            r0 = ((t * NQC + j) * 2 + half) * 65
                            nc.gpsimd.dma_start(out=outc_d[r0:r0 + 65, :],
                                                in_=ost[:, :])
                        del pts[(t, j)]

                # ---- static schedule ----
                # startup projections for head-pair 0, queries/keys 0:512;
                # q first (bf16 q path is the long pole; fp8 k loads fast)
                qk_chunk("q", 0, 0)
                qk_chunk("k", 0, 0)

                # HARD emission deadlines (PE queue is in-order, so a v_st or
                # qk chunk emitted after a PE consumer that semaphore-waits on
                # it would deadlock): v_st(s) before instance first_use(s)+VLAG;
                # qk chunks before the first scores matmul that reads them.
                bg_at = defaultdict(list)
                bg_at[3].append(("qk", "q", 0, 1))   # deadline idx 4
                bg_at[5].append(("qk", "k", 0, 1))   # deadline idx 8
                bg_at[7].append(("qk", "q", 0, 2))   # deadline idx 12
                bg_at[9].append(("qk", "k", 0, 2))   # deadline idx 20
                bg_at[11].append(("qk", "q", 0, 3))  # deadline idx 24
                bg_at[13].append(("qk", "k", 0, 3))  # deadline idx 36
                for s in range(NKT):
                    # v0-7 tight (early vmm deadlines), v8-15 spread later
                    bg_at[4 + 2 * s if s < 8 else 10 + 2 * s].append(("v", s))
                for i, wh, m, sc in (
                        (17, "q", 1, 0), (19, "k", 1, 0), (21, "q", 1, 1),
                        (23, "k", 1, 1), (25, "q", 1, 2), (27, "k", 1, 2),
                        (29, "q", 1, 3), (31, "k", 1, 3),
                        (44, "q", 2, 0), (46, "k", 2, 0), (48, "q", 2, 1),
                        (50, "k", 2, 1), (52, "q", 2, 2), (54, "k", 2, 2),
                        (56, "q", 2, 3), (58, "k", 2, 3),
                        (84, "q", 3, 0), (86, "k", 3, 0), (88, "q", 3, 1),
                        (90, "k", 3, 1), (92, "q", 3, 2), (94, "k", 3, 2),
                        (96, "q", 3, 3), (98, "k", 3, 3)):
                    bg_at[i].append(("qk", wh, m, sc))

                instances = []
                for t in range(NT):
                    # t=3 descending j so the final unit is the smallest
                    # (short tail after the last exp)
                    jorder = (3, 2, 1, 0) if t == NT - 1 else range(NQC)
                    for j in jorder:
                        nkt_u = 4 * (j + 1) if causal else NKT
                        for kt in range(nkt_u):
                            instances.append((t, j, kt))

                vq = deque()

                def drain_vmm(upto):
                    while vq and vq[0][0] <= upto:
                        _, tt, jj, kk, ss = vq.popleft()
                        vmm(tt, jj, kk, ss)

                for idx, (t, j, kt) in enumerate(instances):
                    for item in bg_at.get(idx, ()):
                        if item[0] == "qk":
                            qk_chunk(*item[1:])
                        else:
                            v_st(item[1])
                    drain_vmm(idx - VLAG)
                    slot = idx % NSLOT
                    scores_exp(t, j, kt, slot)
                    vq.append((idx, t, j, kt, slot))
                drain_vmm(10 ** 9)
    _built[causal] = nc
    nc.compile()
    return nc


def _prep_core_inputs(c, q, k, v, Wq, bq, Wk, bk, Wv, bv, sin, cos):
    b, hh = c // 2, c % 2
    hs = slice(hh * DH, (hh + 1) * DH)

    perm = np.empty(DH, np.int64)
    for lh in range(HPC):
        base = (hh * HPC + lh) * HD
        perm[lh * HD:lh * HD + HALF] = base + 2 * np.arange(HALF)
        perm[lh * HD + HALF:(lh + 1) * HD] = base + 2 * np.arange(HALF) + 1

    s = 0.125   # 1/sqrt(HD), folded into the q projection
    S8 = 128.0  # fp8 range scaling for both q/k weights, undone via cs/sn
    wq = np.ascontiguousarray((Wq[perm, :] * (s * S8)).T).astype(BF16)
    wk = np.ascontiguousarray((Wk[perm, :] * S8).T).astype(F8)
    wv = np.ascontiguousarray(Wv[hs, :].T).astype(BF16)

    p32 = np.arange(128) % 32
    cs2 = (cos[:, p32] / S8).T.astype(BF16)
    sgn = np.where((np.arange(128) // 32) % 2 == 0, -1.0, 1.0).astype(np.float32)
    sn2 = (sin[:, p32] * sgn[None, :] / S8).T.astype(BF16)

    kk = np.arange(128)[:, None]
    segs = []
    for d in range(4):
        qq = np.arange(128 * d, 512)[None, :]
        segs.append((128 * d + kk) <= qq)
    msk = np.concatenate(segs, axis=1).astype(BF16)        # [128, 1280]

    bqc = np.ascontiguousarray((bq[perm] * s * S8).reshape(NT, 128).T, np.float32)
    bkc = np.ascontiguousarray((bk[perm] * S8).reshape(NT, 128).T, np.float32)
    bb = np.concatenate([bqc, bkc], axis=1).astype('<f4')  # [128, 8]
    bb16 = np.ascontiguousarray(bb).view('<u2').view(BF16)  # raw halves [128,16]
    msc = np.concatenate([bb16, cs2, sn2], axis=1)

    def pmaj(a):  # [EMB, N] -> [128, NE*N] partition-major slabs
        n = a.shape[1]
        return np.ascontiguousarray(
            a.reshape(NE, 128, n).transpose(1, 0, 2).reshape(128, NE * n))

    def pmaj_m(a):  # [EMB, DH] -> [128, NT*NE*128] m-major slabs
        return np.ascontiguousarray(
            a.reshape(NE, 128, NT, 128).transpose(1, 2, 0, 3).reshape(128, -1))

    def pmaj_sc(a):  # [EMB, SEQ] -> [128, NQC*NE*SC] sc-major slabs
        return np.ascontiguousarray(
            a.reshape(NE, 128, NQC, SC).transpose(1, 2, 0, 3).reshape(128, -1))

    return {
        "xq": pmaj_sc(q[b].T.astype(BF16)),
        "xk": pmaj_sc(k[b].T.astype(F8)),
        "xv": pmaj_sc(v[b].T.astype(BF16)),
        "wq": pmaj_m(wq), "wk": pmaj_m(wk), "wv": pmaj(wv),
        "msc": np.ascontiguousarray(msc), "msk": msk,
    }


def prep_in_maps(q, k, v, Wq, bq, Wk, bk, Wv, bv, sin, cos):
    args = [np.asarray(a, np.float32) for a in (q, k, v, Wq, bq, Wk, bk, Wv, bv, sin, cos)]
    maps = [_prep_core_inputs(c, *args) for c in range(8)]
    return maps, args[8]  # bv needed on host in assemble()


def assemble(results, bv):
    out = np.empty((BATCH, SEQ, EMB), np.float32)
    for c in range(8):
        b, hh = c // 2, c % 2
        oc = np.asarray(results[c]["outc"], np.float32).reshape(NT, NQC, 2, 65, SC)
        an = oc[:, :, :, 0:64, :] / oc[:, :, :, 64:65, :]   # [t, j, half, 64, q]
        an = an.transpose(0, 2, 3, 1, 4).reshape(DH, SEQ)   # [(t,half,d), (j,q)]
        out[b, :, hh * DH:(hh + 1) * DH] = an.T \
            + bv[hh * DH:(hh + 1) * DH][None, :]
    return out


def run(in_maps, causal=True, trace=False, **kw):
    _install_ntff_shim()
    from concourse.bass_utils import run_bass_kernel_spmd
    nc = build(causal)
    return run_bass_kernel_spmd(nc, in_maps, core_ids=list(range(8)), trace=trace, **kw)


def kernel(q, k, v, Wq, bq, Wk, bk, Wv, bv, sin, cos, mask):
    in_maps, bv_f = prep_in_maps(q, k, v, Wq, bq, Wk, bk, Wv, bv, sin, cos)
    r = run(in_maps, causal=bool(mask))
    return assemble(r.results, bv_f)
